# revision 64
# baseline (speedup 1.0000x reference)
"""GRU cell kernel for Trainium2, data-parallel over 8 NeuronCores.

Math (per batch row):
    x_proj = x @ W_ih.T + b           -> r_x, z_x, n_x
    r = sigmoid(r_x + h @ U_r.T)
    z = sigmoid(z_x + h @ U_z.T)
    n = tanh(n_x + r * (h @ U_n.T + U_n_b))
    out = (1 - z) * n + z * h

Layout strategy: all on-chip compute happens in "transposed" orientation so
both matmul operands carry the contraction dim H on the partition axis:
  - host sends x.T, h.T slices per core ([H, B_local]) and pre-packed
    transposed weights; kernel computes out.T tiles [o_feat=128, batch=512]
  - bf16 matmuls (full PE rate), fp32 PSUM accumulation, fp32 epilogue;
    the h used in the final blend is the bf16 copy already resident for
    the matmuls
  - host transposes the per-core [H, B_local] outputs back at the end

Schedule: batch-major block sweep (b0: o0..o7, then b1: o0..o7) with ALL
weights resident in SBUF (12 MiB) + both xh halves (4 MiB).  This halves
the head-of-kernel DMA crunch: before the PE clock finishes ramping only
xh half0 (2 MiB) + w[o0] (1.5 MiB) are needed; xh half1 and w[o2..o7]
trickle in mid-kernel on otherwise-idle queues.  The 16 HBM DMA engines
were measured 100% saturated during the first-o-tile window of the
o-major schedule, so deferring that traffic directly converts PE stall
into work.
"""

import os
import sys
import types

import numpy as np
import ml_dtypes

import concourse.bass as bass
import concourse.mybir as mybir
import concourse.tile as tile
from concourse import bacc
from concourse.bass_utils import run_bass_kernel_spmd


def _ensure_ntff_hook():
    """On images whose ``antenv`` predates ``antenv.axon_hooks``, the traced
    path of ``run_bass_kernel_spmd`` crashes on import (even when tracing is
    merely enabled via the BASS_TRACE env var). Synthesize the module with
    the same ctypes hook the boot code would have registered."""
    try:
        import antenv.axon_hooks  # noqa: F401
        return
    except ImportError:
        pass
    hook = None
    try:
        from trn_agent_boot.trn_boot import _ntff_profile_via_ctypes

        so_path = "/opt/axon/libaxon_pjrt.so"
        if os.path.exists(so_path):
            hook = _ntff_profile_via_ctypes(so_path)
    except Exception:
        hook = None
    mod = types.ModuleType("antenv.axon_hooks")
    mod.get_axon_ntff_profile_hook = lambda: hook
    mod.set_axon_ntff_profile_hook = lambda h: None
    sys.modules["antenv.axon_hooks"] = mod


_ensure_ntff_hook()

H = 1024
B = 8192
NCORES = 8
BL = B // NCORES          # batch rows per core
KT = H // 128             # contraction k-tiles
OT = H // 128             # output-feature tiles (per gate)
NB = BL // 512            # batch slices of 512
F32 = mybir.dt.float32
BF16 = mybir.dt.bfloat16
BF16_NP = ml_dtypes.bfloat16

# gate order inside the packed weight tensor's 768-wide free dim
# g: 0=W_r 1=W_z 2=W_n 3=U_r 4=U_z 5=U_n

LAST_RESULT = None  # BassKernelResults of the most recent run (for test harness)


def _gru_tile_kernel(tc, outt, xh, wp, bias_ap):
    nc = tc.nc
    sig = mybir.ActivationFunctionType.Sigmoid
    tanh = mybir.ActivationFunctionType.Tanh
    add = mybir.AluOpType.add
    mult = mybir.AluOpType.mult

    from contextlib import ExitStack

    with ExitStack() as ctx:
        singles = ctx.enter_context(tc.tile_pool(name="singles", bufs=1))
        gates = ctx.enter_context(tc.tile_pool(name="gates", bufs=3))
        outp = ctx.enter_context(tc.tile_pool(name="outp", bufs=4))
        psum = ctx.enter_context(tc.tile_pool(name="psum", bufs=2, space="PSUM"))

        # resident activations: per (batch-half, k-tile) tiles [x_b | h_b],
        # separate tiles per half so half1's late DMA never gates half0 reads
        xh_t = [
            [
                singles.tile([128, 1024], BF16, name=f"xh{b}_{k}", tag=f"xh{b}_{k}")
                for k in range(KT)
            ]
            for b in range(NB)
        ]
        # ALL weights resident: one [128, kt, 6*128] tile per o-feature tile
        wt = [
            singles.tile([128, KT, 6 * 128], BF16, name=f"wt{o}", tag=f"wt{o}")
            for o in range(OT)
        ]
        bias_t = singles.tile([128, OT * 4], F32, name="bias", tag="bias")
        warm_sb = singles.tile([128, 512], BF16, name="warm_sb", tag="warm_sb")

        xh3 = xh.rearrange("(kt p) b -> kt p b", p=128)
        wv = [wp[o].rearrange("(kt p) f -> p kt f", p=128) for o in range(OT)]

        # ---- head DMA: first-needed-first, one consumption unit per trigger
        # (SWDGE transfers measured ~2x slower than HWDGE and steal engine
        # time, so gpsimd only carries the tiny bias load)
        # gpsimd SWDGE: warm-tile memset (only the 256 columns the warmups
        # read -- a half-size memset finishes ~250ns sooner, and the PE's
        # first LDWEIGHTS is gated on it), then the (tiny) bias load
        nc.gpsimd.memset(warm_sb[:, 0:256], 0.0)
        nc.gpsimd.dma_start(out=bias_t[:], in_=bias_ap[:])
        # scalar HWDGE: w[o0] chunks taper (fine early chunks feed the k-major
        # first block as they land; wider later ones keep more bytes in
        # flight against the shallow ring), then w[o1] k0-3
        for lo, hi in ((0, 1), (1, 2), (2, 4), (4, 6), (6, 8)):
            nc.scalar.dma_start(out=wt[0][:, lo:hi, :], in_=wv[0][:, lo:hi, :])
        # sync HWDGE: xh half0 per k-tile
        for k in range(KT):
            nc.sync.dma_start(out=xh_t[0][k][:], in_=xh3[k][:, 0:1024])
        # w[o1] split across both rings so neither carries >2.75MB early
        for lo, hi in ((0, 2), (2, 4)):
            nc.scalar.dma_start(out=wt[1][:, lo:hi, :], in_=wv[1][:, lo:hi, :])
        for lo, hi in ((4, 6), (6, 8)):
            nc.sync.dma_start(out=wt[1][:, lo:hi, :], in_=wv[1][:, lo:hi, :])

        # warm the PE clock (HAM): the framework preamble ends ~7.2us and the
        # first input chunks land 10.8-13.6us depending on the cold DGE
        # path's mood; the HAM unthrottles the PE clock only after ~3.5us of
        # CONTINUOUS busy and ANY idle resets the ramp, so the dummy matmuls
        # must bridge to the first chunk landing (~12us, past the median
        # landing -- a too-short bridge costs the idle gap PLUS a full clock
        # re-ramp, ~3x the cost of overshooting).  256-row warmups keep each
        # LDWEIGHTS hidden behind the previous matmul while halving the
        # quantization of the bridge's end point.
        warm_ps = psum.tile([128, 512], F32, name="warm_ps", tag="r_ps")
        for _ in range(19):
            nc.tensor.matmul(
                warm_ps[:, 0:256], warm_sb[:, 0:128], warm_sb[:, 0:256],
                start=True, stop=True,
            )

        # deferred loads: everything not needed during the head rides BEHIND
        # real work in its engine's queue so it cannot steal head DMA BW
        # (engines run their queues independently -- only an instruction that
        # follows a data-dependent one is actually paced).  w[o] fires after
        # the r-sigmoid two blocks earlier; xh half1 k-tiles ride the sync
        # ring after mid-b0 store triggers.
        def defer_w(o):
            def fire():
                for lo, hi in ((0, 4), (4, 8)):
                    nc.scalar.dma_start(
                        out=wt[o][:, lo:hi, :], in_=wv[o][:, lo:hi, :]
                    )
            return fire

        def defer_xh1(k):
            def fire():
                nc.sync.dma_start(out=xh_t[1][k][:], in_=xh3[k][:, 1024:2048])
            return fire

        sched_mid = {
            0: [defer_w(2)],
            1: [defer_w(3)],
            2: [defer_w(4)],
            3: [defer_w(5)],
            4: [defer_w(6)],
            5: [defer_w(7)],
        }
        sched_post = {
            1: [defer_xh1(0), defer_xh1(1)],
            2: [defer_xh1(2), defer_xh1(3)],
            3: [defer_xh1(4), defer_xh1(5)],
            4: [defer_xh1(6), defer_xh1(7)],
        }

        xs = bass.ds(0, 512)      # x columns within an xh half-tile
        hs = bass.ds(512, 512)    # h columns

        blocks = [(b, o) for b in range(NB) for o in range(OT)]
        for bi, (b, o) in enumerate(blocks):
            first = bi <= 2   # blocks paced by arriving per-k weight chunks
            last = bi == len(blocks) - 1

            def xap(k, s, w, _b=b):
                return xh_t[_b][k][:, s : s + w]

            # psum tiles are allocated LAZILY, each immediately before its
            # matmul group: allocating all four at block top put all four
            # WAR waits on the block's first matmul (4x53ns of sequencer
            # wait processing > one matmul's 216ns shadow), bubbling the PE
            # at every block boundary; one wait per group start is absorbed
            def ps_tile(name):
                return psum.tile([128, 512], F32, name=name, tag=name)

            def mm_group(ps, parts, c0=0, cw=512):
                n_mm = KT * len(parts)
                i = 0
                for g, cols in parts:
                    for k in range(KT):
                        nc.tensor.matmul(
                            ps[:, c0 : c0 + cw],
                            wt[o][:, k, g * 128 : (g + 1) * 128],
                            xap(k, cols.start + c0, cw),
                            start=(i == 0),
                            stop=(i == n_mm - 1),
                        )
                        i += 1

            if first:
                # paced by the arriving per-k weight/xh chunks: k-major order
                # lets each landing chunk unlock 6 matmuls instead of 1
                # (all four psums allocated up front: these blocks are
                # DMA-paced anyway, the boundary bubble is hidden)
                nx_ps = ps_tile("nx_ps")
                nh_ps = ps_tile("nh_ps")
                r_ps = ps_tile("r_ps")
                z_ps = ps_tile("z_ps")
                km_groups = [
                    (nx_ps, [(2, xs)]),
                    (nh_ps, [(5, hs)]),
                    (r_ps, [(0, xs), (3, hs)]),
                    (z_ps, [(1, xs), (4, hs)]),
                ]
                for k in range(KT):
                    for ps, parts in km_groups:
                        for gi, (g, cols) in enumerate(parts):
                            nc.tensor.matmul(
                                ps[:],
                                wt[o][:, k, g * 128 : (g + 1) * 128],
                                xap(k, cols.start, 512),
                                start=(k == 0 and gi == 0),
                                stop=(k == KT - 1 and gi == len(parts) - 1),
                            )
            elif not last:
                # group order nx, nh, r, z: the n/tanh chain then overlaps
                # the z matmuls, shrinking the per-block tail
                nx_ps = ps_tile("nx_ps")
                mm_group(nx_ps, [(2, xs)])
                nh_ps = ps_tile("nh_ps")
                mm_group(nh_ps, [(5, hs)])
                r_ps = ps_tile("r_ps")
                mm_group(r_ps, [(0, xs), (3, hs)])
            else:
                # final block: r and nh stop early so the serial
                # r-sig -> stt -> add -> tanh -> sub chain finishes while the
                # z matmuls stream; z itself is split by columns (into two
                # SEPARATE psum tiles, so the first group's epilogue is not
                # gated on the second's stop) so the first 384 cols' store is
                # in flight before the last matmul retires
                r_ps = ps_tile("r_ps")
                mm_group(r_ps, [(0, xs), (3, hs)])
                nh_ps = ps_tile("nh_ps")
                mm_group(nh_ps, [(5, hs)])
                nx_ps = ps_tile("nx_ps")
                mm_group(nx_ps, [(2, xs)])

            r_sb = gates.tile([128, 512], F32, name="r", tag="r")
            nc.scalar.activation(
                out=r_sb[:], in_=r_ps[:], func=sig,
                bias=bias_t[:, o * 4 + 0 : o * 4 + 1],
            )
            for fire in sched_mid.get(bi, ()):
                fire()

            zb_ps = None
            if first:
                pass  # z already accumulated in the k-major sweep
            elif not last:
                z_ps = ps_tile("z_ps")
                mm_group(z_ps, [(1, xs), (4, hs)])
            else:
                z_ps = ps_tile("z_ps")
                mm_group(z_ps, [(1, xs), (4, hs)], c0=0, cw=384)

            # t = (n_h + b_n2) * r ; s = n_x + t ; n = tanh(s + b_n1)
            # d = h - n    (all run while the z matmuls stream)
            t_sb = gates.tile([128, 512], F32, name="t", tag="t")
            nc.vector.scalar_tensor_tensor(
                out=t_sb[:], in0=nh_ps[:],
                scalar=bias_t[:, o * 4 + 3 : o * 4 + 4],
                in1=r_sb[:], op0=add, op1=mult,
            )
            s_sb = gates.tile([128, 512], F32, name="s", tag="s")
            nc.vector.tensor_add(s_sb[:], nx_ps[:], t_sb[:])
            n_sb = gates.tile([128, 512], F32, name="n", tag="n")
            nc.scalar.activation(
                out=n_sb[:], in_=s_sb[:], func=tanh,
                bias=bias_t[:, o * 4 + 2 : o * 4 + 3],
            )
            # blend term reads the bf16 h already resident in xh_t
            # (saves the whole 4MB fp32 h stream; z*dh rounding is
            # ~2e-3 of the output scale, well inside tolerance)
            d_sb = gates.tile([128, 512], F32, name="d", tag="d")
            nc.vector.tensor_sub(d_sb[:], xap(o, 512, 512), n_sb[:])

            if last:
                # z cols [384:512] go into a fresh psum tile; the nx tag's
                # other pool buffer was last read a full block ago, so the
                # bank is free
                zb_ps = psum.tile([128, 512], F32, name="zb_ps", tag="nx_ps")
                mm_group(zb_ps, [(1, xs), (4, hs)], c0=384, cw=128)

            # post-z chain in column chunks: z = sigmoid(z_pre + b_z);
            # out = n + z * d, stored as bf16 (half the store bytes and a
            # faster final receipt; the added rounding is ~0.004*|out| --
            # the checker tolerance is 5x that).  Steady-state stores ride
            # the sync HWDGE ring, idle once the xh half0 loads finish.
            z_sb = gates.tile([128, 512], F32, name="z", tag="z")
            p_sb = gates.tile([128, 512], F32, name="p", tag="p")
            o_sb = outp.tile([128, 512], BF16, name="o", tag="o")

            def chain(c0, cw, ps, stores, ew=None):
                cc = bass.ds(c0, cw)
                ew = ew or nc.vector
                nc.scalar.activation(
                    out=z_sb[:, cc], in_=ps[:, cc], func=sig,
                    bias=bias_t[:, o * 4 + 1 : o * 4 + 2],
                )
                ew.tensor_mul(p_sb[:, cc], z_sb[:, cc], d_sb[:, cc])
                ew.tensor_add(o_sb[:, cc], n_sb[:, cc], p_sb[:, cc])
                for st0, sw, eng in stores:
                    eng.dma_start(
                        out=outt[
                            o * 128 : (o + 1) * 128,
                            b * 512 + st0 : b * 512 + st0 + sw,
                        ],
                        in_=o_sb[:, st0 : st0 + sw],
                    )

            if last:
                # 384-col epilogue starts the moment its own z psum stops
                # (one z-subgroup before the final matmul); its vector work
                # is halved so the first store fires early and both rings
                # carry a store trigger in parallel
                chain(0, 192, z_ps, [(0, 192, nc.sync)])
                chain(192, 192, z_ps, [(192, 192, nc.scalar)], ew=nc.gpsimd)
                chain(384, 128, zb_ps, [(384, 128, nc.sync)])
            else:
                chain(0, 256, z_ps, [(0, 256, nc.sync)])
                chain(256, 256, z_ps, [(256, 256, nc.sync)])
            for fire in sched_post.get(bi, ()):
                fire()


_NC_CACHE = None


def _build_nc():
    global _NC_CACHE
    if _NC_CACHE is not None:
        return _NC_CACHE
    nc = bacc.Bacc(
        "TRN2", target_bir_lowering=False, debug=False, num_devices=NCORES
    )
    xh = nc.dram_tensor("xh", [H, 2 * BL], BF16, kind="ExternalInput").ap()
    wp = nc.dram_tensor("wp", [OT, H, 6 * 128], BF16, kind="ExternalInput").ap()
    bias = nc.dram_tensor("bias", [128, OT * 4], F32, kind="ExternalInput").ap()
    outt = nc.dram_tensor("outt", [H, BL], BF16, kind="ExternalOutput").ap()

    with tile.TileContext(nc) as tc:
        _gru_tile_kernel(tc, outt, xh, wp, bias)
    nc.compile()
    _NC_CACHE = nc
    return nc


def _pack_inputs(x, h, W_ih_w, W_ih_b, U_r_w, U_z_w, U_n_w, U_n_b):
    x = np.asarray(x, dtype=np.float32)
    h = np.asarray(h, dtype=np.float32)
    xTb = np.ascontiguousarray(x.T).astype(BF16_NP)     # [H, B]
    hTb = np.ascontiguousarray(h.T).astype(BF16_NP)

    W_all = np.concatenate(
        [np.asarray(W_ih_w, np.float32)] +
        [np.asarray(u, np.float32) for u in (U_r_w, U_z_w, U_n_w)],
        axis=0,
    )                                                   # [6H, H] rows: Wr Wz Wn Ur Uz Un
    WT = np.ascontiguousarray(W_all.T)                  # [H, 6H], col blocks same order
    # wp[o, k, g*128 + m] = WT[k, g*H + o*128 + m]
    wp = np.ascontiguousarray(
        WT.reshape(H, 6, OT, 128).transpose(2, 0, 1, 3).reshape(OT, H, 6 * 128)
    ).astype(BF16_NP)

    b_all = np.concatenate(
        [np.asarray(W_ih_b, np.float32), np.asarray(U_n_b, np.float32)]
    )                                                   # [4H]: b_r b_z b_n1 b_n2
    # bias[m, o*4 + g] = b_all[g*H + o*128 + m]
    bias = np.ascontiguousarray(
        b_all.reshape(4, OT, 128).transpose(2, 1, 0).reshape(128, OT * 4)
    ).astype(np.float32)

    in_maps = []
    for c in range(NCORES):
        sl = slice(c * BL, (c + 1) * BL)
        xc, hc = xTb[:, sl], hTb[:, sl]
        # per-k rows packed as [x_b0 | h_b0 | x_b1 | h_b1]
        xhc = np.concatenate(
            [xc[:, 0:512], hc[:, 0:512], xc[:, 512:1024], hc[:, 512:1024]],
            axis=1,
        )
        in_maps.append({
            "xh": np.ascontiguousarray(xhc),
            "wp": wp,
            "bias": bias,
        })
    return in_maps


def kernel(x, h, W_ih_w, W_ih_b, U_r_w, U_z_w, U_n_w, U_n_b):
    global LAST_RESULT
    nc = _build_nc()
    in_maps = _pack_inputs(x, h, W_ih_w, W_ih_b, U_r_w, U_z_w, U_n_w, U_n_b)
    trace = bool(os.environ.get("GRU_TRACE"))
    res = run_bass_kernel_spmd(nc, in_maps, list(range(NCORES)), trace=trace)
    LAST_RESULT = res
    out = np.empty((B, H), dtype=np.float32)
    for c in range(NCORES):
        out[c * BL : (c + 1) * BL, :] = res.results[c]["outt"].astype(np.float32).T
    return out


# revision 65
# speedup vs baseline: 1.0007x; 1.0007x over previous
"""GRU cell kernel for Trainium2, data-parallel over 8 NeuronCores.

Math (per batch row):
    x_proj = x @ W_ih.T + b           -> r_x, z_x, n_x
    r = sigmoid(r_x + h @ U_r.T)
    z = sigmoid(z_x + h @ U_z.T)
    n = tanh(n_x + r * (h @ U_n.T + U_n_b))
    out = (1 - z) * n + z * h

Layout strategy: all on-chip compute happens in "transposed" orientation so
both matmul operands carry the contraction dim H on the partition axis:
  - host sends x.T, h.T slices per core ([H, B_local]) and pre-packed
    transposed weights; kernel computes out.T tiles [o_feat=128, batch=512]
  - bf16 matmuls (full PE rate), fp32 PSUM accumulation, fp32 epilogue;
    the h used in the final blend is the bf16 copy already resident for
    the matmuls
  - host transposes the per-core [H, B_local] outputs back at the end

Schedule: batch-major block sweep (b0: o0..o7, then b1: o0..o7) with ALL
weights resident in SBUF (12 MiB) + both xh halves (4 MiB).  This halves
the head-of-kernel DMA crunch: before the PE clock finishes ramping only
xh half0 (2 MiB) + w[o0] (1.5 MiB) are needed; xh half1 and w[o2..o7]
trickle in mid-kernel on otherwise-idle queues.  The 16 HBM DMA engines
were measured 100% saturated during the first-o-tile window of the
o-major schedule, so deferring that traffic directly converts PE stall
into work.
"""

import os
import sys
import types

import numpy as np
import ml_dtypes

import concourse.bass as bass
import concourse.mybir as mybir
import concourse.tile as tile
from concourse import bacc
from concourse.bass_utils import run_bass_kernel_spmd


def _ensure_ntff_hook():
    """On images whose ``antenv`` predates ``antenv.axon_hooks``, the traced
    path of ``run_bass_kernel_spmd`` crashes on import (even when tracing is
    merely enabled via the BASS_TRACE env var). Synthesize the module with
    the same ctypes hook the boot code would have registered."""
    try:
        import antenv.axon_hooks  # noqa: F401
        return
    except ImportError:
        pass
    hook = None
    try:
        from trn_agent_boot.trn_boot import _ntff_profile_via_ctypes

        so_path = "/opt/axon/libaxon_pjrt.so"
        if os.path.exists(so_path):
            hook = _ntff_profile_via_ctypes(so_path)
    except Exception:
        hook = None
    mod = types.ModuleType("antenv.axon_hooks")
    mod.get_axon_ntff_profile_hook = lambda: hook
    mod.set_axon_ntff_profile_hook = lambda h: None
    sys.modules["antenv.axon_hooks"] = mod


_ensure_ntff_hook()

H = 1024
B = 8192
NCORES = 8
BL = B // NCORES          # batch rows per core
KT = H // 128             # contraction k-tiles
OT = H // 128             # output-feature tiles (per gate)
NB = BL // 512            # batch slices of 512
F32 = mybir.dt.float32
BF16 = mybir.dt.bfloat16
BF16_NP = ml_dtypes.bfloat16

# gate order inside the packed weight tensor's 768-wide free dim
# g: 0=W_r 1=W_z 2=W_n 3=U_r 4=U_z 5=U_n

LAST_RESULT = None  # BassKernelResults of the most recent run (for test harness)


def _gru_tile_kernel(tc, outt, xh, wp, bias_ap):
    nc = tc.nc
    sig = mybir.ActivationFunctionType.Sigmoid
    tanh = mybir.ActivationFunctionType.Tanh
    add = mybir.AluOpType.add
    mult = mybir.AluOpType.mult

    from contextlib import ExitStack

    with ExitStack() as ctx:
        singles = ctx.enter_context(tc.tile_pool(name="singles", bufs=1))
        gates = ctx.enter_context(tc.tile_pool(name="gates", bufs=3))
        outp = ctx.enter_context(tc.tile_pool(name="outp", bufs=4))
        psum = ctx.enter_context(tc.tile_pool(name="psum", bufs=2, space="PSUM"))

        # resident activations: per (batch-half, k-tile) tiles [x_b | h_b],
        # separate tiles per half so half1's late DMA never gates half0 reads
        xh_t = [
            [
                singles.tile([128, 1024], BF16, name=f"xh{b}_{k}", tag=f"xh{b}_{k}")
                for k in range(KT)
            ]
            for b in range(NB)
        ]
        # ALL weights resident: one [128, kt, 6*128] tile per o-feature tile
        wt = [
            singles.tile([128, KT, 6 * 128], BF16, name=f"wt{o}", tag=f"wt{o}")
            for o in range(OT)
        ]
        bias_t = singles.tile([128, OT * 4], F32, name="bias", tag="bias")
        warm_sb = singles.tile([128, 512], BF16, name="warm_sb", tag="warm_sb")

        xh3 = xh.rearrange("(kt p) b -> kt p b", p=128)
        wv = [wp[o].rearrange("(kt p) f -> p kt f", p=128) for o in range(OT)]

        # ---- head DMA: first-needed-first, one consumption unit per trigger
        # (SWDGE transfers measured ~2x slower than HWDGE and steal engine
        # time, so gpsimd only carries the tiny bias load)
        # gpsimd SWDGE: warm-tile memset (only the 256 columns the warmups
        # read -- a half-size memset finishes ~250ns sooner, and the PE's
        # first LDWEIGHTS is gated on it), then the (tiny) bias load
        nc.gpsimd.memset(warm_sb[:, 0:256], 0.0)
        nc.gpsimd.dma_start(out=bias_t[:], in_=bias_ap[:])
        # scalar HWDGE: w[o0] chunks taper (fine early chunks feed the k-major
        # first block as they land; wider later ones keep more bytes in
        # flight against the shallow ring), then w[o1] k0-3
        for lo, hi in ((0, 1), (1, 2), (2, 4), (4, 6), (6, 8)):
            nc.scalar.dma_start(out=wt[0][:, lo:hi, :], in_=wv[0][:, lo:hi, :])
        # sync HWDGE: xh half0 per k-tile
        for k in range(KT):
            nc.sync.dma_start(out=xh_t[0][k][:], in_=xh3[k][:, 0:1024])
        # w[o1] split across both rings so neither carries >2.75MB early
        for lo, hi in ((0, 2), (2, 4)):
            nc.scalar.dma_start(out=wt[1][:, lo:hi, :], in_=wv[1][:, lo:hi, :])
        for lo, hi in ((4, 6), (6, 8)):
            nc.sync.dma_start(out=wt[1][:, lo:hi, :], in_=wv[1][:, lo:hi, :])

        # warm the PE clock (HAM): the framework preamble ends ~7.2us and the
        # first input chunks land 10.8-13.6us depending on the cold DGE
        # path's mood; the HAM unthrottles the PE clock only after ~3.5us of
        # CONTINUOUS busy and ANY idle resets the ramp, so the dummy matmuls
        # must bridge to the first chunk landing (~12us, past the median
        # landing -- a too-short bridge costs the idle gap PLUS a full clock
        # re-ramp, ~3x the cost of overshooting).  256-row warmups keep each
        # LDWEIGHTS hidden behind the previous matmul while halving the
        # quantization of the bridge's end point.
        warm_ps = psum.tile([128, 512], F32, name="warm_ps", tag="r_ps")
        for _ in range(19):
            nc.tensor.matmul(
                warm_ps[:, 0:256], warm_sb[:, 0:128], warm_sb[:, 0:256],
                start=True, stop=True,
            )

        # deferred loads: everything not needed during the head rides BEHIND
        # real work in its engine's queue so it cannot steal head DMA BW
        # (engines run their queues independently -- only an instruction that
        # follows a data-dependent one is actually paced).  w[o] fires after
        # the r-sigmoid two blocks earlier; xh half1 k-tiles ride the sync
        # ring after mid-b0 store triggers.
        def defer_w(o):
            def fire():
                for lo, hi in ((0, 4), (4, 8)):
                    nc.scalar.dma_start(
                        out=wt[o][:, lo:hi, :], in_=wv[o][:, lo:hi, :]
                    )
            return fire

        def defer_xh1(k):
            def fire():
                nc.sync.dma_start(out=xh_t[1][k][:], in_=xh3[k][:, 1024:2048])
            return fire

        sched_mid = {
            0: [defer_w(2)],
            1: [defer_w(3)],
            2: [defer_w(4)],
            3: [defer_w(5)],
            4: [defer_w(6)],
            5: [defer_w(7)],
        }
        sched_post = {
            1: [defer_xh1(0), defer_xh1(1)],
            2: [defer_xh1(2), defer_xh1(3)],
            3: [defer_xh1(4), defer_xh1(5)],
            4: [defer_xh1(6), defer_xh1(7)],
        }

        xs = bass.ds(0, 512)      # x columns within an xh half-tile
        hs = bass.ds(512, 512)    # h columns

        blocks = [(b, o) for b in range(NB) for o in range(OT)]
        for bi, (b, o) in enumerate(blocks):
            first = bi <= 2   # blocks paced by arriving per-k weight chunks
            last = bi == len(blocks) - 1

            def xap(k, s, w, _b=b):
                return xh_t[_b][k][:, s : s + w]

            # psum tiles are allocated LAZILY, each immediately before its
            # matmul group: allocating all four at block top put all four
            # WAR waits on the block's first matmul (4x53ns of sequencer
            # wait processing > one matmul's 216ns shadow), bubbling the PE
            # at every block boundary; one wait per group start is absorbed
            def ps_tile(name):
                return psum.tile([128, 512], F32, name=name, tag=name)

            def mm_group(ps, parts, c0=0, cw=512):
                n_mm = KT * len(parts)
                i = 0
                for g, cols in parts:
                    for k in range(KT):
                        nc.tensor.matmul(
                            ps[:, c0 : c0 + cw],
                            wt[o][:, k, g * 128 : (g + 1) * 128],
                            xap(k, cols.start + c0, cw),
                            start=(i == 0),
                            stop=(i == n_mm - 1),
                        )
                        i += 1

            if first:
                # paced by the arriving per-k weight/xh chunks: k-major order
                # lets each landing chunk unlock 6 matmuls instead of 1
                # (all four psums allocated up front: these blocks are
                # DMA-paced anyway, the boundary bubble is hidden)
                nx_ps = ps_tile("nx_ps")
                nh_ps = ps_tile("nh_ps")
                r_ps = ps_tile("r_ps")
                z_ps = ps_tile("z_ps")
                km_groups = [
                    (nx_ps, [(2, xs)]),
                    (nh_ps, [(5, hs)]),
                    (r_ps, [(0, xs), (3, hs)]),
                    (z_ps, [(1, xs), (4, hs)]),
                ]
                for k in range(KT):
                    for ps, parts in km_groups:
                        for gi, (g, cols) in enumerate(parts):
                            nc.tensor.matmul(
                                ps[:],
                                wt[o][:, k, g * 128 : (g + 1) * 128],
                                xap(k, cols.start, 512),
                                start=(k == 0 and gi == 0),
                                stop=(k == KT - 1 and gi == len(parts) - 1),
                            )
            elif not last:
                # group order nx, nh, r, z: the n/tanh chain then overlaps
                # the z matmuls, shrinking the per-block tail
                nx_ps = ps_tile("nx_ps")
                mm_group(nx_ps, [(2, xs)])
                nh_ps = ps_tile("nh_ps")
                mm_group(nh_ps, [(5, hs)])
                r_ps = ps_tile("r_ps")
                mm_group(r_ps, [(0, xs), (3, hs)])
            else:
                # final block: r and nh stop early so the serial
                # r-sig -> stt -> add -> tanh -> sub chain finishes while the
                # z matmuls stream; z itself is split by columns (into two
                # SEPARATE psum tiles, so the first group's epilogue is not
                # gated on the second's stop) so the first 384 cols' store is
                # in flight before the last matmul retires
                r_ps = ps_tile("r_ps")
                mm_group(r_ps, [(0, xs), (3, hs)])
                nh_ps = ps_tile("nh_ps")
                mm_group(nh_ps, [(5, hs)])
                nx_ps = ps_tile("nx_ps")
                mm_group(nx_ps, [(2, xs)])

            r_sb = gates.tile([128, 512], F32, name="r", tag="r")
            nc.scalar.activation(
                out=r_sb[:], in_=r_ps[:], func=sig,
                bias=bias_t[:, o * 4 + 0 : o * 4 + 1],
            )
            for fire in sched_mid.get(bi, ()):
                fire()

            zb_ps = None
            if first:
                pass  # z already accumulated in the k-major sweep
            elif not last:
                z_ps = ps_tile("z_ps")
                mm_group(z_ps, [(1, xs), (4, hs)])
            else:
                z_ps = ps_tile("z_ps")
                mm_group(z_ps, [(1, xs), (4, hs)], c0=0, cw=384)

            # t = (n_h + b_n2) * r ; s = n_x + t ; n = tanh(s + b_n1)
            # d = h - n    (all run while the z matmuls stream)
            t_sb = gates.tile([128, 512], F32, name="t", tag="t")
            nc.vector.scalar_tensor_tensor(
                out=t_sb[:], in0=nh_ps[:],
                scalar=bias_t[:, o * 4 + 3 : o * 4 + 4],
                in1=r_sb[:], op0=add, op1=mult,
            )
            s_sb = gates.tile([128, 512], F32, name="s", tag="s")
            nc.vector.tensor_add(s_sb[:], nx_ps[:], t_sb[:])
            n_sb = gates.tile([128, 512], F32, name="n", tag="n")
            nc.scalar.activation(
                out=n_sb[:], in_=s_sb[:], func=tanh,
                bias=bias_t[:, o * 4 + 2 : o * 4 + 3],
            )
            # blend term reads the bf16 h already resident in xh_t
            # (saves the whole 4MB fp32 h stream; z*dh rounding is
            # ~2e-3 of the output scale, well inside tolerance)
            d_sb = gates.tile([128, 512], F32, name="d", tag="d")
            nc.vector.tensor_sub(d_sb[:], xap(o, 512, 512), n_sb[:])

            if last:
                # z cols [384:512] go into a fresh psum tile; the nx tag's
                # other pool buffer was last read a full block ago, so the
                # bank is free
                zb_ps = psum.tile([128, 512], F32, name="zb_ps", tag="nx_ps")
                mm_group(zb_ps, [(1, xs), (4, hs)], c0=384, cw=128)

            # post-z chain in column chunks: z = sigmoid(z_pre + b_z);
            # out = n + z * d, stored as bf16 (half the store bytes and a
            # faster final receipt; the added rounding is ~0.004*|out| --
            # the checker tolerance is 5x that).  Steady-state stores ride
            # the sync HWDGE ring, idle once the xh half0 loads finish.
            z_sb = gates.tile([128, 512], F32, name="z", tag="z")
            p_sb = gates.tile([128, 512], F32, name="p", tag="p")
            o_sb = outp.tile([128, 512], BF16, name="o", tag="o")

            def chain(c0, cw, ps, stores, ew=None):
                cc = bass.ds(c0, cw)
                ew = ew or nc.vector
                nc.scalar.activation(
                    out=z_sb[:, cc], in_=ps[:, cc], func=sig,
                    bias=bias_t[:, o * 4 + 1 : o * 4 + 2],
                )
                ew.tensor_mul(p_sb[:, cc], z_sb[:, cc], d_sb[:, cc])
                ew.tensor_add(o_sb[:, cc], n_sb[:, cc], p_sb[:, cc])
                for st0, sw, eng in stores:
                    eng.dma_start(
                        out=outt[
                            o * 128 : (o + 1) * 128,
                            b * 512 + st0 : b * 512 + st0 + sw,
                        ],
                        in_=o_sb[:, st0 : st0 + sw],
                    )

            if last:
                # 384-col epilogue starts the moment its own z psum stops
                # (one z-subgroup before the final matmul); its vector work
                # is halved so the first store fires early and both rings
                # carry a store trigger in parallel
                chain(0, 192, z_ps, [(0, 192, nc.sync)])
                chain(192, 192, z_ps, [(192, 192, nc.scalar)], ew=nc.gpsimd)
                # the very last store splits across both rings: two 16KB
                # transfers with parallel trigger issue retire ~0.3us
                # sooner than one 32KB store on a single ring
                chain(384, 128, zb_ps, [(384, 64, nc.sync), (448, 64, nc.scalar)])
            else:
                chain(0, 256, z_ps, [(0, 256, nc.sync)])
                chain(256, 256, z_ps, [(256, 256, nc.sync)])
            for fire in sched_post.get(bi, ()):
                fire()


_NC_CACHE = None


def _build_nc():
    global _NC_CACHE
    if _NC_CACHE is not None:
        return _NC_CACHE
    nc = bacc.Bacc(
        "TRN2", target_bir_lowering=False, debug=False, num_devices=NCORES
    )
    xh = nc.dram_tensor("xh", [H, 2 * BL], BF16, kind="ExternalInput").ap()
    wp = nc.dram_tensor("wp", [OT, H, 6 * 128], BF16, kind="ExternalInput").ap()
    bias = nc.dram_tensor("bias", [128, OT * 4], F32, kind="ExternalInput").ap()
    outt = nc.dram_tensor("outt", [H, BL], BF16, kind="ExternalOutput").ap()

    with tile.TileContext(nc) as tc:
        _gru_tile_kernel(tc, outt, xh, wp, bias)
    nc.compile()
    _NC_CACHE = nc
    return nc


def _pack_inputs(x, h, W_ih_w, W_ih_b, U_r_w, U_z_w, U_n_w, U_n_b):
    x = np.asarray(x, dtype=np.float32)
    h = np.asarray(h, dtype=np.float32)
    xTb = np.ascontiguousarray(x.T).astype(BF16_NP)     # [H, B]
    hTb = np.ascontiguousarray(h.T).astype(BF16_NP)

    W_all = np.concatenate(
        [np.asarray(W_ih_w, np.float32)] +
        [np.asarray(u, np.float32) for u in (U_r_w, U_z_w, U_n_w)],
        axis=0,
    )                                                   # [6H, H] rows: Wr Wz Wn Ur Uz Un
    WT = np.ascontiguousarray(W_all.T)                  # [H, 6H], col blocks same order
    # wp[o, k, g*128 + m] = WT[k, g*H + o*128 + m]
    wp = np.ascontiguousarray(
        WT.reshape(H, 6, OT, 128).transpose(2, 0, 1, 3).reshape(OT, H, 6 * 128)
    ).astype(BF16_NP)

    b_all = np.concatenate(
        [np.asarray(W_ih_b, np.float32), np.asarray(U_n_b, np.float32)]
    )                                                   # [4H]: b_r b_z b_n1 b_n2
    # bias[m, o*4 + g] = b_all[g*H + o*128 + m]
    bias = np.ascontiguousarray(
        b_all.reshape(4, OT, 128).transpose(2, 1, 0).reshape(128, OT * 4)
    ).astype(np.float32)

    in_maps = []
    for c in range(NCORES):
        sl = slice(c * BL, (c + 1) * BL)
        xc, hc = xTb[:, sl], hTb[:, sl]
        # per-k rows packed as [x_b0 | h_b0 | x_b1 | h_b1]
        xhc = np.concatenate(
            [xc[:, 0:512], hc[:, 0:512], xc[:, 512:1024], hc[:, 512:1024]],
            axis=1,
        )
        in_maps.append({
            "xh": np.ascontiguousarray(xhc),
            "wp": wp,
            "bias": bias,
        })
    return in_maps


def kernel(x, h, W_ih_w, W_ih_b, U_r_w, U_z_w, U_n_w, U_n_b):
    global LAST_RESULT
    nc = _build_nc()
    in_maps = _pack_inputs(x, h, W_ih_w, W_ih_b, U_r_w, U_z_w, U_n_w, U_n_b)
    trace = bool(os.environ.get("GRU_TRACE"))
    res = run_bass_kernel_spmd(nc, in_maps, list(range(NCORES)), trace=trace)
    LAST_RESULT = res
    out = np.empty((B, H), dtype=np.float32)
    for c in range(NCORES):
        out[c * BL : (c + 1) * BL, :] = res.results[c]["outt"].astype(np.float32).T
    return out
